# revision 10
# baseline (speedup 1.0000x reference)
"""RNNT decoder kernel for TRN2 — 8-core SPMD, T-sharded joint, replicated LSTM.

v3: fp8 DoubleRow recurrent/input matmuls (weights pre-scaled x64, undone via
activation scale=1/64), software-pipelined interleave of L0/L1/X1/joint so the
tensor engine stays saturated, contiguous 256KB output DMAs.

Layouts (feature dim on partitions):
  c0/c1        [128, (kc4, b4)] f32
  gates psum   [128, (mc16, b4)] f32      gate blocks reordered (i, f, o, g~)
  X0/X1        [128, (u64, mc16, b4)] f32 input projections + bias, all x64
  H08/H18      [128, (kc4, u64, b4)] fp8  h history (rhs for DR matmuls)
  hencT        [128, (jc4, b4, t32)] f32
  hdecJT       [128, (jc4, u64, b4)] f32
  zT bf16      [128, (jc4, u8, b4, t32)]  per u-block
  out psum     [128o, (u4, b4, t32)]      contiguous DMA to DRAM
"""

import numpy as np
import ml_dtypes

import concourse.bass as bass
import concourse.mybir as mybir
import concourse.tile as tile
from concourse import bacc
from concourse import bass_utils
from concourse.masks import make_identity

B, T, U, E, H, J, OD, G = 4, 256, 64, 512, 512, 512, 1024, 2048
NCORES = 8
TLOC = T // NCORES          # 32
UBLK = 8
NBLK = U // UBLK            # 8
F32 = mybir.dt.float32
BF16 = mybir.dt.bfloat16
FP8 = mybir.dt.float8e4
I32 = mybir.dt.int32
AF = mybir.ActivationFunctionType
DR = mybir.MatmulPerfMode.DoubleRow
BF = ml_dtypes.bfloat16
F8NP = mybir.dt.np(FP8)
WS = 64.0                   # fp8 weight pre-scale

_CACHE = {}


def _build():
    nc = bacc.Bacc("TRN2", target_bir_lowering=False, debug=False,
                   enable_asserts=False, num_devices=NCORES)
    hs = nc.dram_tensor("hs", [B, TLOC, E], BF16, kind="ExternalInput").ap()
    emb = nc.dram_tensor("emb", [1024, E], FP8, kind="ExternalInput").ap()
    idx = nc.dram_tensor("idx", [B * U], I32, kind="ExternalInput").ap()
    whh0 = nc.dram_tensor("whh0", [H, G], FP8, kind="ExternalInput").ap()
    wih0 = nc.dram_tensor("wih0", [E, G], FP8, kind="ExternalInput").ap()
    whh1 = nc.dram_tensor("whh1", [H, G], FP8, kind="ExternalInput").ap()
    wih1 = nc.dram_tensor("wih1", [H, G], FP8, kind="ExternalInput").ap()
    wenc = nc.dram_tensor("wenc", [E, J], BF16, kind="ExternalInput").ap()
    wdec = nc.dram_tensor("wdec", [H, J], FP8, kind="ExternalInput").ap()
    wout = nc.dram_tensor("wout", [J, OD], BF16, kind="ExternalInput").ap()
    b0 = nc.dram_tensor("b0", [G], F32, kind="ExternalInput").ap()
    b1 = nc.dram_tensor("b1", [G], F32, kind="ExternalInput").ap()
    benc = nc.dram_tensor("benc", [J], F32, kind="ExternalInput").ap()
    bout = nc.dram_tensor("bout", [OD], F32, kind="ExternalInput").ap()
    # device-native order: [oc, ub, hf, p, u, b, t]; host un-permutes.
    yout = nc.dram_tensor("out", [8, NBLK, 2, 128, UBLK // 2, B, TLOC], F32,
                          kind="ExternalOutput").ap()

    from contextlib import ExitStack
    with tile.TileContext(nc) as tc, ExitStack() as ctx:
        P = ctx.enter_context(tc.tile_pool(name="persist", bufs=1))
        WK = ctx.enter_context(tc.tile_pool(name="work", bufs=3))
        DBL = ctx.enter_context(tc.tile_pool(name="dbl", bufs=2))
        PA = ctx.enter_context(tc.tile_pool(name="pa", bufs=2, space="PSUM"))
        PG0 = ctx.enter_context(tc.tile_pool(name="pg0", bufs=2, space="PSUM"))
        PG1 = ctx.enter_context(tc.tile_pool(name="pg1", bufs=2, space="PSUM"))
        PJ = ctx.enter_context(tc.tile_pool(name="pj", bufs=2, space="PSUM"))

        # ---- weight loads (pre-transposed on host: [K, M*] contiguous) ----
        def load_w(name, dram, width, dt):
            t = P.tile([128, 4 * width], dt, tag=name)
            nc.sync.dma_start(t[:].rearrange("p (kc j) -> p kc j", kc=4),
                              dram.rearrange("(kc p) j -> p kc j", p=128))
            return t
        whh08 = load_w("whh08", whh0, G, FP8)
        whh18 = load_w("whh18", whh1, G, FP8)
        wih08 = load_w("wih08", wih0, G, FP8)
        wih18 = load_w("wih18", wih1, G, FP8)
        wencT = load_w("wencT", wenc, J, BF16)
        wdec8 = load_w("wdec8", wdec, J, FP8)
        woutT = load_w("woutT", wout, OD, BF16)

        # ---- biases ----
        b0T = P.tile([128, 16], F32, tag="b0T")
        nc.sync.dma_start(b0T[:], b0.rearrange("(mc p) -> p mc", p=128))
        b1T = P.tile([128, 16], F32, tag="b1T")
        nc.sync.dma_start(b1T[:], b1.rearrange("(mc p) -> p mc", p=128))
        bencT = P.tile([128, 4], F32, tag="bencT")
        nc.sync.dma_start(bencT[:], benc.rearrange("(jc p) -> p jc", p=128))
        boutT = P.tile([128, 8], F32, tag="boutT")
        nc.sync.dma_start(boutT[:], bout.rearrange("(oc p) -> p oc", p=128))

        # ---- embedding gather + transpose -> eysT8 [128, (ec4, u64, b4)] ----
        idx_sb = P.tile([128, 2], I32, tag="idx")
        for r in range(2):
            nc.sync.dma_start(idx_sb[:, r:r + 1], idx[r * 128:(r + 1) * 128].unsqueeze(1))
        identb = P.tile([128, 128], BF16, tag="identb")
        make_identity(nc, identb[:])
        eysT8 = P.tile([128, 4 * 256], FP8, tag="eysT8")
        for r in range(2):
            eys_sb = WK.tile([128, E], FP8, tag="eys_sb")
            nc.gpsimd.indirect_dma_start(
                out=eys_sb[:], out_offset=None, in_=emb,
                in_offset=bass.IndirectOffsetOnAxis(ap=idx_sb[:, r:r + 1], axis=0))
            eys_bf = WK.tile([128, E], BF16, tag="eys_bf")
            nc.vector.tensor_copy(eys_bf[:], eys_sb[:])
            for ec in range(4):
                pst = PA.tile([128, 128], BF16, tag="pa")
                nc.tensor.transpose(out=pst[:], in_=eys_bf[:, ec * 128:(ec + 1) * 128],
                                    identity=identb[:])
                nc.vector.tensor_copy(eysT8[:, ec * 256 + r * 128: ec * 256 + r * 128 + 128],
                                      pst[:])

        # ---- hs slice -> hsT [128, (ec, b, t)] bf16 ----
        hs_sb = P.tile([128, E], BF16, tag="hs_sb")
        for b in range(B):
            nc.sync.dma_start(hs_sb[b * TLOC:(b + 1) * TLOC, :], hs[b])
        hsT = P.tile([128, 4 * 128], BF16, tag="hsT")
        for ec in range(4):
            pst = PA.tile([128, 128], BF16, tag="pa")
            nc.tensor.transpose(out=pst[:], in_=hs_sb[:, ec * 128:(ec + 1) * 128],
                                identity=identb[:])
            nc.vector.tensor_copy(hsT[:, ec * 128:(ec + 1) * 128], pst[:])

        # ---- henc -> hencT [128, (jc, b, t)] f32 ----
        hencT = P.tile([128, 4 * 128], F32, tag="hencT")
        for jc in range(4):
            ps = PA.tile([128, 128], F32, tag="pa")
            for kc in range(4):
                nc.tensor.matmul(
                    ps[:], lhsT=wencT[:, kc * J + jc * 128: kc * J + jc * 128 + 128],
                    rhs=hsT[:, kc * 128:(kc + 1) * 128],
                    start=(kc == 0), stop=(kc == 3))
            nc.vector.tensor_scalar_add(hencT[:, jc * 128:(jc + 1) * 128], ps[:],
                                        bencT[:, jc:jc + 1])

        # fp8 weight views [128, kc, j]
        wih08v = wih08[:].rearrange("p (kc j) -> p kc j", kc=4)
        wih18v = wih18[:].rearrange("p (kc j) -> p kc j", kc=4)
        whh08v = whh08[:].rearrange("p (kc j) -> p kc j", kc=4)
        whh18v = whh18[:].rearrange("p (kc j) -> p kc j", kc=4)
        wdec8v = wdec8[:].rearrange("p (kc j) -> p kc j", kc=4)
        eysT8v = eysT8[:].rearrange("p (ec n) -> p ec n", ec=4)

        # ---- X0 = (64*Wih0)@(64*eys) -> x64 scale via evict ----
        X0 = P.tile([128, U * 64], F32, tag="X0")
        X0v = X0[:].rearrange("p (u mc b) -> p u mc b", u=U, mc=16)
        for mc in range(16):
            px = PA.tile([128, 256], F32, tag="pa")
            for kcp in range(2):
                nc.tensor.matmul(
                    px[:],
                    lhsT=wih08v[:, 2 * kcp:2 * kcp + 2, mc * 128:(mc + 1) * 128],
                    rhs=eysT8v[:, 2 * kcp:2 * kcp + 2, :],
                    start=(kcp == 0), stop=(kcp == 1), perf_mode=DR)
            nc.vector.scalar_tensor_tensor(
                X0v[:, :, mc, :],
                px[:].rearrange("p (u b) -> p u b", u=U),
                1.0 / WS,
                b0T[:, mc:mc + 1].unsqueeze(1).to_broadcast([128, U, 4]),
                op0=mybir.AluOpType.mult, op1=mybir.AluOpType.add)

        # ---- persistent state ----
        X1 = P.tile([128, U * 64], F32, tag="X1")
        X1v = X1[:].rearrange("p (u mc b) -> p u mc b", u=U, mc=16)
        H08 = P.tile([128, 4 * U * B], FP8, tag="H08")
        H18 = P.tile([128, 4 * U * B], FP8, tag="H18")
        H08v = H08[:].rearrange("p (kc u b) -> p kc u b", kc=4, u=U)
        H18v = H18[:].rearrange("p (kc u b) -> p kc u b", kc=4, u=U)
        hdecJT = P.tile([128, 4 * 256], F32, tag="hdecJT")
        hdecJTv = hdecJT[:].rearrange("p (jc u b) -> p jc u b", jc=4, u=U)
        c0 = P.tile([128, 16], F32, tag="c0")
        nc.vector.memset(c0[:], 0.0)
        c1 = P.tile([128, 16], F32, tag="c1")
        nc.vector.memset(c1[:], 0.0)

        def lstm_step(u, X, whhv, Hv, c, PG, lt):
            """One LSTM step: DR gate matmuls + activations; h -> Hv[:, :, u, :] fp8."""
            Xu = X[:, u * 64:(u + 1) * 64]
            if u == 0:
                g = Xu
            else:
                ps = PG.tile([128, 64], F32, tag=f"g{lt}")
                for mc in range(16):
                    for kcp in range(2):
                        nc.tensor.matmul(
                            ps[:, mc * 4:(mc + 1) * 4],
                            lhsT=whhv[:, 2 * kcp:2 * kcp + 2, mc * 128:(mc + 1) * 128],
                            rhs=Hv[:, 2 * kcp:2 * kcp + 2, u - 1, :],
                            start=(kcp == 0), stop=(kcp == 1), perf_mode=DR)
                g_sb = WK.tile([128, 64], F32, tag=f"g_sb{lt}")
                nc.vector.tensor_add(g_sb[:], ps[:], Xu)
                g = g_sb
            # gate order: i (0:16), f (16:32), o (32:48), g~ (48:64); all x64
            s_if = WK.tile([128, 32], F32, tag=f"s_if{lt}")
            nc.scalar.activation(s_if[:], g[:, 0:32], AF.Sigmoid, scale=1.0 / WS)
            t_g = WK.tile([128, 16], F32, tag=f"t_g{lt}")
            nc.scalar.activation(t_g[:], g[:, 48:64], AF.Tanh, scale=1.0 / WS)
            s_o = WK.tile([128, 16], F32, tag=f"s_o{lt}")
            nc.scalar.activation(s_o[:], g[:, 32:48], AF.Sigmoid, scale=1.0 / WS)
            if u == 0:
                nc.vector.tensor_mul(c[:], s_if[:, 0:16], t_g[:])
            else:
                t1 = WK.tile([128, 16], F32, tag=f"t1{lt}")
                nc.vector.tensor_mul(t1[:], s_if[:, 16:32], c[:])
                t2 = WK.tile([128, 16], F32, tag=f"t2{lt}")
                nc.vector.tensor_mul(t2[:], s_if[:, 0:16], t_g[:])
                nc.vector.tensor_add(c[:], t1[:], t2[:])
            t_c = WK.tile([128, 16], F32, tag=f"t_c{lt}")
            nc.scalar.activation(t_c[:], c[:], AF.Tanh)
            nc.vector.tensor_mul(
                Hv[:, :, u, :],
                s_o[:].rearrange("p (kc b) -> p kc b", kc=4),
                t_c[:].rearrange("p (kc b) -> p kc b", kc=4))

        def x1_proj(b):
            """X1 block b = (64*Wih1)@h0_fp8 + 64*b1."""
            for mc in range(16):
                px = PA.tile([128, 32], F32, tag="pa")
                for kcp in range(2):
                    nc.tensor.matmul(
                        px[:],
                        lhsT=wih18v[:, 2 * kcp:2 * kcp + 2, mc * 128:(mc + 1) * 128],
                        rhs=H08v[:, 2 * kcp:2 * kcp + 2, UBLK * b:UBLK * (b + 1), :],
                        start=(kcp == 0), stop=(kcp == 1), perf_mode=DR)
                nc.vector.scalar_tensor_tensor(
                    X1v[:, UBLK * b:UBLK * (b + 1), mc, :],
                    px[:].rearrange("p (u b) -> p u b", u=UBLK),
                    1.0,
                    b1T[:, mc:mc + 1].unsqueeze(1).to_broadcast([128, UBLK, 4]),
                    op0=mybir.AluOpType.mult, op1=mybir.AluOpType.add)

        def hdec_proj(b):
            """hdecJT block b = Wdec@h1 (psum is x64; evict scales back)."""
            for jc in range(4):
                pd = PA.tile([128, 32], F32, tag="pa")
                for kcp in range(2):
                    nc.tensor.matmul(
                        pd[:],
                        lhsT=wdec8v[:, 2 * kcp:2 * kcp + 2, jc * 128:(jc + 1) * 128],
                        rhs=H18v[:, 2 * kcp:2 * kcp + 2, UBLK * b:UBLK * (b + 1), :],
                        start=(kcp == 0), stop=(kcp == 1), perf_mode=DR)
                nc.scalar.activation(
                    hdecJTv[:, jc, UBLK * b:UBLK * (b + 1), :],
                    pd[:].rearrange("p (u b) -> p u b", u=UBLK),
                    AF.Copy, scale=1.0 / WS)

        zT_tiles = {}

        def joint_st1(ub):
            """z = tanh(henc + hdec) for u-block ub -> zT bf16."""
            zT = DBL.tile([128, 4 * UBLK * B * TLOC], BF16, tag="zT")
            zT_tiles[ub] = zT
            for jc in range(4):
                zin = DBL.tile([128, UBLK * B * TLOC], F32, tag="zin")
                henc_bc = (hencT[:, jc * 128:(jc + 1) * 128]
                           .rearrange("p (b t) -> p b t", b=B)
                           .unsqueeze(1).to_broadcast([128, UBLK, B, TLOC]))
                hdec_bc = (hdecJT[:, jc * 256 + ub * UBLK * B: jc * 256 + (ub + 1) * UBLK * B]
                           .rearrange("p (u b) -> p u b", u=UBLK)
                           .unsqueeze(3).to_broadcast([128, UBLK, B, TLOC]))
                nc.vector.tensor_add(
                    zin[:].rearrange("p (u b t) -> p u b t", u=UBLK, b=B),
                    henc_bc, hdec_bc)
                nc.scalar.activation(zT[:, jc * 1024:(jc + 1) * 1024], zin[:], AF.Tanh)

        def joint_st2(ub):
            """out = zT @ Wout.T + bout for u-block ub; contiguous DMA."""
            zT = zT_tiles.pop(ub)
            for oc in range(8):
                for hf in range(2):
                    ps = PJ.tile([128, 512], F32, tag="jout")
                    for jc in range(4):
                        nc.tensor.matmul(
                            ps[:],
                            lhsT=woutT[:, jc * OD + oc * 128: jc * OD + oc * 128 + 128],
                            rhs=zT[:, jc * 1024 + hf * 512: jc * 1024 + hf * 512 + 512],
                            start=(jc == 0), stop=(jc == 3))
                    zout = DBL.tile([128, 512], F32, tag="zout")
                    nc.vector.tensor_scalar_add(zout[:], ps[:], boutT[:, oc:oc + 1])
                    nc.sync.dma_start(
                        yout[oc, ub, hf],
                        zout[:].rearrange("p (u b t) -> p u b t", u=UBLK // 2, b=B))

        # ---- software-pipelined main loop ----
        for s in range(U + UBLK):
            if s < U:
                lstm_step(s, X0, whh08v, H08v, c0, PG0, 0)
            if s >= UBLK:
                lstm_step(s - UBLK, X1, whh18v, H18v, c1, PG1, 1)
            if (s + 1) % UBLK == 0:
                k = (s + 1) // UBLK       # 1..9
                if k - 1 < NBLK:
                    x1_proj(k - 1)        # L0 block k-1 just completed
                if 0 <= k - 2 < NBLK:
                    hdec_proj(k - 2)      # L1 block k-2 just completed
                    joint_st1(k - 2)
                if 0 <= k - 3 < NBLK:
                    joint_st2(k - 3)
        joint_st2(NBLK - 1)
    nc.compile()
    return nc


def _get_nc():
    if "nc" not in _CACHE:
        _CACHE["nc"] = _build()
    return _CACHE["nc"]


# torch gate order (i, f, g, o) -> device order (i, f, o, g~)
_PERM = np.concatenate([np.arange(0, 512), np.arange(512, 1024),
                        np.arange(1536, 2048), np.arange(1024, 1536)])


def _prep_w8(w):
    """[2048, 512] f32 -> [512, 2048] fp8, gate-permuted, x64."""
    return np.ascontiguousarray(
        np.asarray(w, np.float32)[_PERM].T * WS).astype(F8NP)


def kernel(**inputs):
    nc = _get_nc()
    hs_pad = np.asarray(inputs["hs_pad"], np.float32)
    ys_pad = np.asarray(inputs["ys_pad"])
    embed = np.asarray(inputs["embed"], np.float32)

    ys_in = np.concatenate([np.zeros((B, 1), ys_pad.dtype), ys_pad], axis=1)
    idx = np.ascontiguousarray(ys_in.T).reshape(-1).astype(np.int32)  # u-major

    common = {
        "emb": (embed * WS).astype(F8NP),
        "idx": idx,
        "whh0": _prep_w8(inputs["W_hh0"]),
        "wih0": _prep_w8(inputs["W_ih0"]),
        "whh1": _prep_w8(inputs["W_hh1"]),
        "wih1": _prep_w8(inputs["W_ih1"]),
        "wenc": np.ascontiguousarray(np.asarray(inputs["W_enc"], np.float32).T).astype(BF),
        "wdec": np.ascontiguousarray(
            np.asarray(inputs["W_dec"], np.float32).T * WS).astype(F8NP),
        "wout": np.ascontiguousarray(np.asarray(inputs["W_out"], np.float32).T).astype(BF),
        "b0": ((np.asarray(inputs["b_ih0"], np.float32)
                + np.asarray(inputs["b_hh0"], np.float32))[_PERM] * WS).copy(),
        "b1": ((np.asarray(inputs["b_ih1"], np.float32)
                + np.asarray(inputs["b_hh1"], np.float32))[_PERM] * WS).copy(),
        "benc": np.asarray(inputs["b_enc"], np.float32),
        "bout": np.asarray(inputs["b_out"], np.float32),
    }
    in_maps = []
    for c in range(NCORES):
        m = dict(common)
        m["hs"] = np.ascontiguousarray(
            hs_pad[:, c * TLOC:(c + 1) * TLOC, :]).astype(BF)
        in_maps.append(m)

    _CACHE["in_maps"] = in_maps
    res = bass_utils.run_bass_kernel_spmd(nc, in_maps, core_ids=list(range(NCORES)))
    outs = []
    for r in res.results:
        o = np.asarray(r["out"]).reshape(8, NBLK, 2, 128, UBLK // 2, B, TLOC)
        outs.append(np.transpose(o, (5, 6, 1, 2, 4, 0, 3)).reshape(B, TLOC, U, OD))
    return np.concatenate(outs, axis=1).astype(np.float32)
